# revision 15
# baseline (speedup 1.0000x reference)
"""Event-RGB dynamic fusion module on 8 trn2 NeuronCores (v4).

Per-pixel dynamic 3x3 depthwise kernels predicted from concat(rgb, event)
via two 1x1 convs + relu, applied to reflect-padded rgb.

Sharding: 8 shards = (batch b in 0..3) x (H half in {0,1}); each core gets
reflect-padded rgb slabs (two bf16 copies at element offsets 0/1 so every
3x3-shift view stays 4-byte aligned for DVE 2x mode), a bf16 event slab,
and replicated pre-laid-out bf16 weights. Fully data-parallel, no
collectives.

PE side (same as v1 baseline — M=64 column-disjoint tiles reach ~2.9x
concurrent streams on the PE, which M=128 variants cannot):
  mm1 (K=96 via 64+32 accum, M=128) -> h4 psum, relu+b1 on ACT -> h4 bf16
  mm2: 9 taps x [K=32, M=64] tiles at (32*rg, 64*hf), emitted round-robin
  in groups of 3 taps so streams overlap.

Apply side (new vs v1):
  - taps paired into [128,2048] bf16 tiles; products are single 2x-mode
    TTs against overlapping 4D patch views (AP surgery).
  - O2 pair product runs on GPSIMD (started first), E/O1 on DVE.
  - O3 pair + center tap are direct 1x STTs from PSUM on DVE (no ACT
    copy), queued before the copied products so DVE never stalls on ACT.
  - add tree on [2048] tiles hits DVE 4x mode; the V/Wf/out tail of each
    sub-slice is emitted one sub-slice late (software pipelining) so the
    DVE queue never waits on the GPSIMD partial sum.
  - output is written bf16 and widened to fp32 on the host.
"""

import os
from contextlib import ExitStack

import ml_dtypes
import numpy as np

import bass_rust
import concourse.bass as bass
import concourse.bacc as bacc
import concourse.mybir as mybir
import concourse.tile as tile
from concourse.bass_utils import run_bass_kernel_spmd

B, C, H, W = 4, 64, 256, 256
CEV, KK, MID = 32, 3, 32
NCORES = 8
SHARD_H = 128          # rows per core
HALF = 64              # rows per half (partition-packing of pixel halves)
RBLK = 16              # rows per half per block
NBLK = HALF // RBLK    # 4
WE = 260               # padded row length (even, so shifted views stay aligned)
SUBR = 4               # rows per half per sub-slice (=1024 px per half)
NSUB = RBLK // SUBR    # 4
F32 = mybir.dt.float32
BF16 = mybir.dt.bfloat16
AOP = mybir.AluOpType
RELU = mybir.ActivationFunctionType.Relu
IDENT = mybir.ActivationFunctionType.Identity
BF = ml_dtypes.bfloat16

# tap index ij = 3*(di+1) + (dj+1); emission slots: 3 groups of 3 taps;
# rg = slot % 4 keeps row-groups distinct inside each group.
TAP_SLOTS = [3, 5, 6, 8, 4, 1, 7, 0, 2]
TAP_RG = {ij: s % 4 for s, ij in enumerate(TAP_SLOTS)}
# copied pairs: (name, [ij_a, ij_b], slab, tap_stride_elems, row_off, col)
PAIRS = {
    "O2": ([3, 5], "o", 2, 0, 0),        # (0,-1) & (0,+1)
    "E": ([1, 7], "e", 2 * WE, -1, 2),   # (-1,0) & (+1,0)
    "O1": ([0, 2], "o", 2, -1, 0),       # (-1,-1) & (-1,+1)
}
O3_TAPS = [(6, 0), (8, 2)]               # direct-STT pair (+1,-1),(+1,+1)
S_TAP = 4                                # direct-STT center tap

_cache = {}


def _pair_view(flat_ap, pstride, tap_stride, base):
    """Overlapping [128, 2, 4, 256] view of a flat [128, N] slab tile."""
    v = flat_ap.copy()
    v.ap = bass_rust.VecI64Pair(
        [[pstride, 128], [tap_stride, 2], [WE, SUBR], [1, W]])
    v.offset = flat_ap.offset + base
    return v


def _build():
    nc = bacc.Bacc("TRN2", target_bir_lowering=False, debug=False)
    rgbe = nc.dram_tensor("rgbe", [C, SHARD_H + 2, WE], BF16, kind="ExternalInput").ap()
    rgbo = nc.dram_tensor("rgbo", [C, SHARD_H + 2, WE], BF16, kind="ExternalInput").ap()
    ev = nc.dram_tensor("ev", [CEV, SHARD_H, W], BF16, kind="ExternalInput").ap()
    w1 = nc.dram_tensor("w1", [128, 384], BF16, kind="ExternalInput").ap()
    w2 = nc.dram_tensor("w2", [128, 9 * 128], BF16, kind="ExternalInput").ap()
    bi = nc.dram_tensor("bi", [128, 10], F32, kind="ExternalInput").ap()
    out = nc.dram_tensor("out", [C, SHARD_H, W], BF16, kind="ExternalOutput").ap()

    with tile.TileContext(nc) as tc, ExitStack() as ctx:
        _kernel(ctx, tc, rgbe, rgbo, ev, w1, w2, bi, out)
    nc.compile()
    return nc


def _kernel(ctx, tc, rgbe, rgbo, ev, w1, w2, bi, out):
    nc = tc.nc
    consts = ctx.enter_context(tc.tile_pool(name="consts", bufs=1))
    rgb_p = ctx.enter_context(tc.tile_pool(name="rgb", bufs=2))
    ev_p = ctx.enter_context(tc.tile_pool(name="evp", bufs=2))
    h4_p = ctx.enter_context(tc.tile_pool(name="h4", bufs=2))
    dkb_p = ctx.enter_context(tc.tile_pool(name="dkb", bufs=4))
    sprod_p = ctx.enter_context(tc.tile_pool(name="sprod", bufs=3))
    prod_p = ctx.enter_context(tc.tile_pool(name="prod", bufs=6))
    acc_p = ctx.enter_context(tc.tile_pool(name="acc", bufs=5))
    outt_p = ctx.enter_context(tc.tile_pool(name="outt", bufs=4))
    ph_p = ctx.enter_context(tc.tile_pool(name="psum_h", bufs=2, space="PSUM"))
    pdk_p = ctx.enter_context(tc.tile_pool(name="psum_dk", bufs=3, space="PSUM"))

    w1t = consts.tile([128, 384], BF16)
    nc.sync.dma_start(w1t[:], w1[:])
    w2t = consts.tile([128, 9 * 128], BF16)
    nc.sync.dma_start(w2t[:], w2[:])
    bt = consts.tile([128, 10], F32)
    nc.sync.dma_start(bt[:], bi[:])

    npx = RBLK * W           # pixels per half per block (4096)
    pend = [None]            # software-pipelined tail state

    for t in range(NBLK):
        rge = rgb_p.tile([128, (RBLK + 2) * WE], BF16, tag="rge")
        nc.sync.dma_start(rge[0:64, :], rgbe[:, t * RBLK:t * RBLK + RBLK + 2, :])
        nc.sync.dma_start(
            rge[64:128, :], rgbe[:, HALF + t * RBLK:HALF + t * RBLK + RBLK + 2, :])
        rgo = rgb_p.tile([128, (RBLK + 2) * WE], BF16, tag="rgo")
        nc.sync.dma_start(rgo[0:64, :], rgbo[:, t * RBLK:t * RBLK + RBLK + 2, :])
        nc.sync.dma_start(
            rgo[64:128, :], rgbo[:, HALF + t * RBLK:HALF + t * RBLK + RBLK + 2, :])
        evt = ev_p.tile([128, RBLK * W], BF16)
        nc.sync.dma_start(evt[64:96, :], ev[:, t * RBLK:t * RBLK + RBLK, :])
        nc.sync.dma_start(
            evt[96:128, :], ev[:, HALF + t * RBLK:HALF + t * RBLK + RBLK, :])

        rgev = rge[:].rearrange("p (r w) -> p r w", w=WE)      # [128, 18, 260]
        rgov = rgo[:].rearrange("p (r w) -> p r w", w=WE)
        evv = evt[:].rearrange("p (r w) -> p r w", w=W)        # [128, 16, 256]
        pstride_e = rge[:].ap[0][0]
        pstride_o = rgo[:].ap[0][0]

        # ---- mm1 for the whole block: h4[32q+m, px] x4 copies ----
        h4 = h4_p.tile([128, 2 * npx], BF16)
        for sl in range(RBLK // 2):              # 512-px slices per half
            r0 = 2 * sl
            ph = ph_p.tile([128, 512], F32, tag="ph")
            ph2 = ph_p.tile([128, 512], F32, tag="ph")
            nc.tensor.matmul(ph[:], w1t[0:64, 0:128],
                             rgev[0:64, r0 + 1:r0 + 3, 2:258],
                             start=True, stop=False, tile_position=(0, 0))
            nc.tensor.matmul(ph2[:], w1t[64:128, 128:256],
                             rgev[64:128, r0 + 1:r0 + 3, 2:258],
                             start=True, stop=False, tile_position=(64, 0))
            nc.tensor.matmul(ph[:], w1t[64:96, 0:128],
                             evv[64:96, r0:r0 + 2, :],
                             start=False, stop=True, tile_position=(64, 0))
            nc.tensor.matmul(ph2[:], w1t[96:128, 256:384],
                             evv[96:128, r0:r0 + 2, :],
                             start=False, stop=True, tile_position=(96, 0))
            nc.scalar.activation(h4[:, 512 * sl:512 * (sl + 1)], ph[:],
                                 RELU, bias=bt[:, 0:1], scale=1.0)
            nc.scalar.activation(h4[:, npx + 512 * sl:npx + 512 * (sl + 1)],
                                 ph2[:], RELU, bias=bt[:, 0:1], scale=1.0)

        def tadd(key, shape, a, b, pool=acc_p, gps=False):
            r = pool.tile(shape, BF16, tag="acc", name=f"acc{key}")
            eng = nc.gpsimd if gps else nc.vector
            eng.tensor_tensor(r[:], a, b, op=AOP.add)
            return r

        def pair_product(name, dks, r0):
            ijs, slab, tstride, drow, bcol = PAIRS[name]
            dkb = dkb_p.tile([128, 2048], BF16)
            for hfi, ij in enumerate(ijs):
                nc.scalar.activation(dkb[:, 1024 * hfi:1024 * hfi + 1024],
                                     dks[ij][:], IDENT,
                                     bias=bt[:, 1 + ij:2 + ij], scale=1.0)
            flat = rge[:] if slab == "e" else rgo[:]
            pstr = pstride_e if slab == "e" else pstride_o
            patch = _pair_view(flat, pstr, tstride, (r0 + 1 + drow) * WE + bcol)
            prod = prod_p.tile([128, 2048], BF16)
            dkbv = dkb[:].rearrange("p (t r w) -> p t r w", t=2, w=W)
            prodv = prod[:].rearrange("p (t r w) -> p t r w", t=2, w=W)
            eng = nc.gpsimd if name == "O2" else nc.vector
            eng.tensor_tensor(prodv[:], dkbv[:], patch, op=AOP.mult)
            return prod

        def o3_stt(po3, k, dks, r0):
            ij, bcol = O3_TAPS[k]
            pv = po3[:, 1024 * k:1024 * k + 1024].rearrange(
                "p (r w) -> p r w", w=W)
            nc.vector.scalar_tensor_tensor(
                pv[:], dks[ij][:], bt[:, 1 + ij:2 + ij],
                rgov[:, r0 + 2:r0 + 6, bcol:bcol + 256],
                op0=AOP.add, op1=AOP.mult)

        def flush_tail(st):
            tV = tadd("V", [128, 2048], st["T"][:], st["U"][:])
            tW = tadd("Wf", [128, 1024], tV[:, 0:1024], tV[:, 1024:2048])
            ot = tadd("out", [128, 1024], tW[:], st["S"][:], pool=outt_p)
            otv = ot[:].rearrange("p (r w) -> p r w", w=W)
            ra = st["ra"]
            nc.sync.dma_start(out[:, ra:ra + SUBR, :], otv[0:64, :, :])
            nc.sync.dma_start(out[:, HALF + ra:HALF + ra + SUBR, :],
                              otv[64:128, :, :])

        for s in range(NSUB):
            r0 = SUBR * s

            # ---- mm2: 3 groups of 3 taps, 12 matmuls per group emitted
            # round-robin over the taps (distinct rg/hf tiles -> the PE
            # streams up to ~3 of them concurrently).
            dks = {}
            for g in range(3):
                ijs = TAP_SLOTS[3 * g:3 * g + 3]
                for ij in ijs:
                    dks[ij] = pdk_p.tile([128, 1024], F32,
                                         name=f"dk{ij}", tag="dk")
                for hf in range(2):
                    for nh in range(2):
                        for ij in ijs:
                            rg = TAP_RG[ij]
                            hc0 = npx * hf + 1024 * s + 512 * nh
                            lh = w2t[32 * rg:32 * rg + 32,
                                     128 * TAP_SLOTS.index(ij) + 64 * hf:
                                     128 * TAP_SLOTS.index(ij) + 64 * hf + 64]
                            nc.tensor.matmul(
                                dks[ij][64 * hf:64 * hf + 64,
                                        512 * nh:512 * nh + 512],
                                lh, h4[32 * rg:32 * rg + 32, hc0:hc0 + 512],
                                start=True, stop=True,
                                tile_position=(32 * rg, 64 * hf))
                # apply work for the taps this group completes:
                if g == 0:
                    # O2 pair done -> ACT copies + GPSIMD product (earliest)
                    prod_o2 = pair_product("O2", dks, r0)
                    # O3a direct STT
                    po3 = prod_p.tile([128, 2048], BF16, name="prodO3")
                    o3_stt(po3, 0, dks, r0)
                elif g == 1:
                    o3_stt(po3, 1, dks, r0)
                    spv_ = sprod_p.tile([128, 1024], BF16)
                    sv = spv_[:].rearrange("p (r w) -> p r w", w=W)
                    nc.vector.scalar_tensor_tensor(
                        sv[:], dks[S_TAP][:], bt[:, 1 + S_TAP:2 + S_TAP],
                        rgev[:, r0 + 1:r0 + 5, 2:258],
                        op0=AOP.add, op1=AOP.mult)
                else:
                    prod_e = pair_product("E", dks, r0)
                    prod_o1 = pair_product("O1", dks, r0)

            tU = tadd("U", [128, 2048], prod_o2[:], po3[:], gps=True)
            tT = tadd("T", [128, 2048], prod_e[:], prod_o1[:])
            if pend[0] is not None:
                flush_tail(pend[0])
            pend[0] = {"T": tT, "U": tU, "S": spv_, "ra": t * RBLK + r0}

    flush_tail(pend[0])


def _prep_consts(W1, b1, W2, b2):
    W1T = np.ascontiguousarray(W1.T)                              # [96, 32]
    W1T4 = np.tile(W1T, (1, 4))                                   # [96, 128]
    w1sb = np.zeros((128, 384), np.float32)
    w1sb[0:64, 0:128] = W1T4[0:64]          # rgb A
    w1sb[64:96, 0:128] = W1T4[64:96]        # ev A
    w1sb[64:128, 128:256] = W1T4[0:64]      # rgb B
    w1sb[96:128, 256:384] = W1T4[64:96]     # ev B

    W2r = W2.reshape(C, 9, MID)
    w2sb = np.zeros((128, 9 * 128), np.float32)
    for slot, ij in enumerate(TAP_SLOTS):
        rg = slot % 4
        wij = np.ascontiguousarray(W2r[:, ij, :].T)               # [32, 64]
        w2sb[32 * rg:32 * rg + 32, 128 * slot:128 * slot + 64] = wij
        w2sb[32 * rg:32 * rg + 32, 128 * slot + 64:128 * slot + 128] = wij

    bisb = np.zeros((128, 10), np.float32)
    bisb[:, 0] = np.tile(b1, 4)
    b2r = b2.reshape(C, 9)
    for ij in range(9):
        bisb[:, 1 + ij] = np.concatenate([b2r[:, ij], b2r[:, ij]])
    return w1sb.astype(BF), w2sb.astype(BF), bisb


def _shard_inputs(rgb_feature, event_feature, W1, b1, W2, b2):
    rgbp = np.pad(rgb_feature, ((0, 0), (0, 0), (1, 1), (1, 1)), mode="reflect")
    rgbe = np.zeros((B, C, H + 2, WE), BF)
    rgbo = np.zeros((B, C, H + 2, WE), BF)
    rgbe[:, :, :, 1:1 + W + 2] = rgbp
    rgbo[:, :, :, 0:W + 2] = rgbp
    evb = event_feature.astype(BF)
    w1sb, w2sb, bisb = _prep_consts(W1, b1, W2, b2)
    in_maps = []
    for k in range(NCORES):
        b, r0 = k // 2, SHARD_H * (k % 2)
        in_maps.append({
            "rgbe": np.ascontiguousarray(rgbe[b, :, r0:r0 + SHARD_H + 2, :]),
            "rgbo": np.ascontiguousarray(rgbo[b, :, r0:r0 + SHARD_H + 2, :]),
            "ev": np.ascontiguousarray(evb[b, :, r0:r0 + SHARD_H, :]),
            "w1": w1sb, "w2": w2sb, "bi": bisb,
        })
    return in_maps


def _run(inputs, trace=False, **trace_kwargs):
    if "nc" not in _cache:
        _cache["nc"] = _build()
    nc = _cache["nc"]
    in_maps = _shard_inputs(
        inputs["rgb_feature"].astype(np.float32),
        inputs["event_feature"].astype(np.float32),
        inputs["W1"].astype(np.float32), inputs["b1"].astype(np.float32),
        inputs["W2"].astype(np.float32), inputs["b2"].astype(np.float32))
    res = run_bass_kernel_spmd(nc, in_maps, list(range(NCORES)),
                               trace=trace, **trace_kwargs)
    full = np.empty((B, C, H, W), np.float32)
    for k in range(NCORES):
        b, r0 = k // 2, SHARD_H * (k % 2)
        full[b, :, r0:r0 + SHARD_H, :] = res.results[k]["out"].astype(np.float32)
    return full, res


def kernel(**inputs):
    full, _ = _run(inputs, trace=False)
    return full
